# revision 22
# baseline (speedup 1.0000x reference)
"""CARCell (CARFAC cascade) Trainium2 Bass kernel.

Strategy
--------
Data-parallel over batch: 32 rows -> 8 NeuronCores x 4 rows. Per core the
T=8192 time recurrence is sequential; per step everything lives in a
[C=84 partitions, B=4 free] layout so the nonlinear-damping chain runs as
~10 tiny VectorE ops with per-partition-scalar channel constants, and the
sequential 84-channel cascade is reformulated as constant-weight matmuls:

    y(t)  = Wy_ru @ ru + Wy_rv @ rv + G * inp          (output cascade)
    u(t)  = Wu_ru @ ru + Wu_rv @ rv + wu_inp * inp     (cascade + rotation)
    v(t)  = c0 * ru + a0 * rv                          (elementwise)

with L[i,j] = prod_{k=j..i} g0[k] the cascade propagation matrix,
precomputed on the host in float64. The u-update matmul runs per step
(state carried in PSUM); the y matmul is batched over 32 steps
(lhsT = stored [ru;inp] block of shape [85, 32*4] -> out [128, 84] in PSUM)
and DMA'd PSUM->DRAM. Audio enters via one DMA per 32-step block straight
into partition 84 of the stored block, so no per-step engine copy is needed.

Host-side numpy does all layout shuffles (transposes are free there).
"""
import os
from contextlib import ExitStack

import numpy as np

import concourse.bass as bass
import concourse.tile as tile
from concourse import mybir
from concourse.bass import ds
from concourse.bass_utils import run_bass_kernel_spmd

F32 = mybir.dt.float32
ALU = mybir.AluOpType
AF = mybir.ActivationFunctionType

N_CORES = 8
B_TOT, T, C = 32, 8192, 84
BL = B_TOT // N_CORES          # 4 batch rows per core
SB = 32                        # steps per y-matmul block
NJ = 4                         # y-blocks per loop body
BS = SB * NJ                   # 128 steps per loop body
NB = T // BS                   # 64 loop iterations
SAMPLE_RATE_HZ = 48000.0


# ----------------------------------------------------------------- host math
def _coeffs(hfdc, zero_ratio, min_zeta, max_zeta, erb_break, erb_q):
    # pole_freqs with float32 stepping to mirror the jax f32 scan exactly
    pf = np.empty(C, np.float64)
    f = np.float32(20400.0)
    eb = np.float32(erb_break)
    eq = np.float32(erb_q)
    half = np.float32(0.5)
    for i in range(C):
        pf[i] = f
        f = np.float32(f - half * (eb + f) / eq)
    hfdc, zero_ratio, min_zeta, max_zeta, erb_break, erb_q = (
        float(hfdc), float(zero_ratio), float(min_zeta), float(max_zeta),
        float(erb_break), float(erb_q))
    f_ratio = zero_ratio ** 2 - 1.0
    x = pf * 2.0 / SAMPLE_RATE_HZ
    theta = x * np.pi
    a0 = np.cos(theta)
    c0 = np.sin(theta)
    zr1 = np.pi * (x - hfdc * x ** 3)
    r1 = 1.0 - zr1 * max_zeta
    erb_pf = (erb_break + pf) / erb_q
    min_zetas = min_zeta + 0.25 * (erb_pf / pf - min_zeta)
    zr = zr1 * (max_zeta - min_zetas)
    rpz = r1 + zr
    h = c0 * f_ratio
    g0 = (1.0 - 2.0 * rpz * a0 + rpz ** 2) / (
        1.0 - 2.0 * rpz * a0 + h * rpz * c0 + rpz ** 2)
    return a0, c0, r1, zr, h, g0


def _weights(a0, c0, h, g0):
    L = np.zeros((C, C))
    for i in range(C):
        L[i, i] = g0[i]
        if i:
            L[i, :i] = g0[i] * L[i - 1, :i]
    G = L[:, 0].copy()
    hc0 = h * c0
    ha0 = h * a0
    Wy_ru = np.tril(L) * hc0[None, :]
    Wy_rv = np.tril(L) * ha0[None, :]
    SL = np.zeros((C, C))
    SL[1:, :] = np.tril(L)[:-1, :]
    Wu_ru = np.diag(a0) + SL * hc0[None, :]
    Wu_rv = np.diag(-c0) + SL * ha0[None, :]
    wu_inp = np.empty(C)
    wu_inp[0] = 1.0
    wu_inp[1:] = G[:-1]
    # lhsT layouts [K, M]: K = [ru(0:84); inp(84)], M = out channel
    wu1t = np.zeros((C + 1, C), np.float32)
    wu1t[:C, :] = Wu_ru.T
    wu1t[C, :] = wu_inp
    wu2t = np.zeros((C + 1, C), np.float32)
    wu2t[:C, :] = Wu_rv.T
    # y-matmul rhs layouts [K, N=C]
    wy1t = np.zeros((C + 1, C), np.float32)
    wy1t[:C, :] = Wy_ru.T
    wy1t[C, :] = G
    wy2t = np.zeros((C + 1, C), np.float32)
    wy2t[:C, :] = Wy_rv.T
    return wu1t, wu2t, wy1t, wy2t


# ------------------------------------------------------------- device program
_CACHE = {}


def _build_program(legalize=True):
    if "nc" in _CACHE:
        return _CACHE["nc"]
    nc = bass.Bass("TRN2", target_bir_lowering=False, debug=False,
                   num_devices=N_CORES)
    aud = nc.dram_tensor("aud", [NB, NJ, SB * BL], F32,
                         kind="ExternalInput").ap()
    # packed constants: [85, 4*84 + 6 + 12 + 8] = weights | vecs | state0 | bcast
    NCON = 4 * C + 6 + 12 + 2 * BL
    con = nc.dram_tensor("con", [C + 1, NCON], F32, kind="ExternalInput").ap()
    yout = nc.dram_tensor("yout", [NB, NJ, SB * BL, C], F32,
                          kind="ExternalOutput").ap()

    with tile.TileContext(nc) as tc, ExitStack() as ctx:
        const = ctx.enter_context(tc.tile_pool(name="const", bufs=1))
        stp = ctx.enter_context(tc.tile_pool(name="stp", bufs=1))
        temps = ctx.enter_context(tc.tile_pool(name="temps", bufs=2))
        blkp = ctx.enter_context(tc.tile_pool(name="blkp", bufs=2))
        rvp = ctx.enter_context(tc.tile_pool(name="rvp", bufs=2))
        ups = ctx.enter_context(tc.tile_pool(name="ups", bufs=1, space="PSUM"))
        yps = ctx.enter_context(tc.tile_pool(name="yps", bufs=2, space="PSUM"))
        ysbp = ctx.enter_context(tc.tile_pool(name="ysbp", bufs=2))

        c_sb = const.tile([C + 1, NCON], F32)
        nc.sync.dma_start(c_sb[:, :], con)

        wu1t = c_sb[:, 0 * C:1 * C]
        wu2t = c_sb[0:C, 1 * C:2 * C]
        wy1t = c_sb[:, 2 * C:3 * C]
        wy2t = c_sb[0:C, 3 * C:4 * C]
        vbase = 4 * C
        vs_ap = c_sb[0:C, vbase + 0:vbase + 1]
        vo_ap = c_sb[0:C, vbase + 1:vbase + 2]
        zr_ap = c_sb[0:C, vbase + 2:vbase + 3]
        r1_ap = c_sb[0:C, vbase + 3:vbase + 4]
        c0_ap = c_sb[0:C, vbase + 4:vbase + 5]
        a0_ap = c_sb[0:C, vbase + 5:vbase + 6]
        sbase = vbase + 6
        s_sb = c_sb[0:C, sbase:sbase + 12]
        bbase = sbase + 12
        b_sb = c_sb[0:C, bbase:bbase + 2 * BL]

        u_ps = [ups.tile([C, BL], F32, name="u_ps0", tag="u_ps0"),
                ups.tile([C, BL], F32, name="u_ps1", tag="u_ps1")]
        v_sl = [stp.tile([C, BL], F32, name="v_sl0", tag="v_sl0"),
                stp.tile([C, BL], F32, name="v_sl1", tag="v_sl1")]
        # step s reads u(t-1) from u_ps[(s+1)%2], v(t-1) from v_sl[(s+1)%2],
        # v(t-2) from v_sl[s%2]; writes u(t)->u_ps[s%2], v(t)->v_sl[s%2].
        nc.vector.tensor_copy(u_ps[1][:, :], s_sb[:, 0:4])
        nc.vector.tensor_copy(v_sl[1][:, :], s_sb[:, 4:8])
        nc.vector.tensor_copy(v_sl[0][:, :], s_sb[:, 8:12])

        for i in range(NB):
            for j in range(NJ):
                ruext = blkp.tile([C + 1, SB, BL], F32, name="ruext", tag="ruext")
                rvst = rvp.tile([C, SB, BL], F32, name="rvst", tag="rvst")
                # audio for this 32-step block -> partition 84 of ruext
                nc.sync.dma_start(ruext[C:C + 1, :, :], aud[i, j, :])
                for ss in range(SB):
                    s = j * SB + ss
                    vp = v_sl[(s + 1) % 2]   # v(t-1)
                    vpp = v_sl[s % 2]        # v(t-2)
                    up = u_ps[(s + 1) % 2]   # u(t-1)
                    uw = u_ps[s % 2]
                    vel = temps.tile([C, BL], F32, name="vel", tag="vel")
                    nc.vector.tensor_sub(vel[:, :], vp[:, :], vpp[:, :])
                    q = temps.tile([C, BL], F32, name="q", tag="q")
                    nc.scalar.activation(q[:, :], vel[:, :], AF.Square,
                                         bias=vo_ap, scale=vs_ap)
                    den = temps.tile([C, BL], F32, name="den", tag="den")
                    nc.scalar.activation(den[:, :], q[:, :], AF.Identity,
                                         bias=1.0, scale=1.0)
                    rec = temps.tile([C, BL], F32, name="rec", tag="rec")
                    nc.vector.reciprocal(rec[:, :], den[:, :])
                    r = temps.tile([C, BL], F32, name="r", tag="r")
                    nc.scalar.activation(r[:, :], rec[:, :], AF.Identity,
                                         bias=r1_ap, scale=zr_ap)
                    ru = ruext[0:C, ss, :]
                    nc.vector.tensor_mul(ru, r[:, :], up[:, :])
                    rv = rvst[:, ss, :]
                    nc.vector.tensor_mul(rv, r[:, :], vp[:, :])
                    t9 = temps.tile([C, BL], F32, name="t9", tag="t9")
                    nc.scalar.activation(t9[:, :], ru, AF.Copy,
                                         bias=0.0, scale=c0_ap)
                    # v(t) = a0*rv + c0*ru, overwrites v(t-2) slot
                    nc.vector.scalar_tensor_tensor(vpp[:, :], rv, a0_ap,
                                                   t9[:, :], ALU.mult,
                                                   ALU.add)
                    nc.tensor.matmul(uw[:, :], wu1t, ruext[:, ss, :],
                                     start=True, stop=False)
                    nc.tensor.matmul(uw[:, :], wu2t, rv,
                                     start=False, stop=True)
                yb = yps.tile([SB * BL, C], F32, name="yb", tag="yb")
                nc.tensor.matmul(yb[:, :], ruext[:, :, :], wy1t,
                                 start=True, stop=False)
                nc.tensor.matmul(yb[:, :], rvst[:, :, :], wy2t,
                                 start=False, stop=True)
                ysb = ysbp.tile([SB * BL, C], F32, name="ysb", tag="ysb")
                nc.scalar.copy(ysb[:, :], yb[:, :])
                nc.sync.dma_start(yout[i, j, :, :], ysb[:, :])

    if legalize:
        _legalize_waits(nc)
    _CACHE["nc"] = nc
    return nc


def _legalize_waits(nc, max_waits=2):
    """Walrus encodes at most 2 sync-wait commands per instruction; offload
    excess waits onto same-engine NoOps inserted right before (engine queues
    are in-order, so the NoOp's wait gates everything after it)."""
    import bass_rust
    nid = [0]
    for f in nc.m.functions:
        for blk in f.blocks:
            out = []
            changed = False
            for ins in blk.instructions:
                si = ins.sync_info
                waits = list(si.on_wait) if si is not None else []
                max_waits = 1
                if len(waits) > max_waits:
                    # keep the last max_waits (self-sem + closest producer);
                    # offload older ones, one per NoOp to stay conservative.
                    excess, keep = waits[:-max_waits], waits[-max_waits:]
                    for w in excess:
                        nop = mybir.InstNoOp(name=f"waitnop_{nid[0]}")
                        nid[0] += 1
                        nop.engine = ins.engine
                        nop.sync_info = bass_rust.SyncInfo(
                            on_wait=[w], on_update=[])
                        out.append(nop)
                    ins.sync_info = bass_rust.SyncInfo(
                        on_wait=keep, on_update=list(si.on_update))
                    changed = True
                out.append(ins)
            if changed:
                blk.instructions = out


# ------------------------------------------------------------------ interface
def _pack_inputs(audio, u0, v0, pv0, hfdc, zero_ratio, min_zeta, max_zeta,
                 erb_break, erb_q, v_offset, velocity_scale):
    a0, c0, r1, zr, h, g0 = _coeffs(hfdc, zero_ratio, min_zeta, max_zeta,
                                    erb_break, erb_q)
    wu1t, wu2t, wy1t, wy2t = _weights(a0, c0, h, g0)
    wts = np.concatenate([wu1t, wu2t, wy1t, wy2t], axis=1)  # [C+1, 4C]
    vecs = np.zeros((C + 1, 6), np.float32)
    vecs[:C] = np.stack([
        np.full(C, float(velocity_scale)),
        np.full(C, float(v_offset)),
        zr, r1, c0, a0,
    ], axis=1).astype(np.float32)
    bcast = np.zeros((C + 1, 2 * BL), np.float32)
    bcast[:C] = np.concatenate([
        np.full((C, BL), float(v_offset)),
        np.broadcast_to(r1[:, None], (C, BL)),
    ], axis=1).astype(np.float32)

    in_maps = []
    for k in range(N_CORES):
        bsl = slice(k * BL, (k + 1) * BL)
        a = np.ascontiguousarray(audio[bsl], np.float32)   # [BL, T]
        aud = np.ascontiguousarray(
            a.T.reshape(NB, NJ, SB, BL).reshape(NB, NJ, SB * BL))
        st = np.zeros((C + 1, 12), np.float32)
        st[:C] = np.concatenate([u0[bsl].T, v0[bsl].T, pv0[bsl].T],
                                axis=1).astype(np.float32)
        con = np.concatenate([wts.astype(np.float32), vecs, st, bcast],
                             axis=1)                       # [C+1, NCON]
        in_maps.append({
            "aud": aud,
            "con": np.ascontiguousarray(con),
        })
    return in_maps


def _unpack_output(results):
    out = np.empty((B_TOT, T, C), np.float32)
    for k in range(N_CORES):
        y = results[k]["yout"]                       # [NB, NJ, SB*BL, C]
        y = y.reshape(NB, NJ, SB, BL, C).transpose(3, 0, 1, 2, 4)
        out[k * BL:(k + 1) * BL] = y.reshape(BL, T, C)
    return out


def run(inputs_kw, trace=False):
    nc = _build_program()
    in_maps = _pack_inputs(
        np.asarray(inputs_kw["audio"], np.float32),
        np.asarray(inputs_kw["u0"], np.float32),
        np.asarray(inputs_kw["v0"], np.float32),
        np.asarray(inputs_kw["pv0"], np.float32),
        inputs_kw["high_f_damping_compression"], inputs_kw["zero_ratio"],
        inputs_kw["min_zeta"], inputs_kw["max_zeta"],
        inputs_kw["erb_break_freq"], inputs_kw["erb_q"],
        inputs_kw["v_offset"], inputs_kw["velocity_scale"])
    res = run_bass_kernel_spmd(nc, in_maps, list(range(N_CORES)),
                               trace=trace)
    return _unpack_output(res.results), res


def kernel(**inputs):
    out, _ = run(inputs, trace=False)
    return out


# revision 24
# speedup vs baseline: 1.3309x; 1.3309x over previous
"""CARCell (CARFAC cascade) Trainium2 Bass kernel.

Strategy
--------
Data-parallel over batch: 32 rows -> 8 NeuronCores x 4 rows. Per core the
T=8192 time recurrence is sequential; per step everything lives in a
[C=84 partitions, B=4 free] layout so the nonlinear-damping chain runs as
~10 tiny VectorE ops with per-partition-scalar channel constants, and the
sequential 84-channel cascade is reformulated as constant-weight matmuls:

    y(t)  = Wy_ru @ ru + Wy_rv @ rv + G * inp          (output cascade)
    u(t)  = Wu_ru @ ru + Wu_rv @ rv + wu_inp * inp     (cascade + rotation)
    v(t)  = c0 * ru + a0 * rv                          (elementwise)

with L[i,j] = prod_{k=j..i} g0[k] the cascade propagation matrix,
precomputed on the host in float64. The u-update matmul runs per step
(state carried in PSUM); the y matmul is batched over 32 steps
(lhsT = stored [ru;inp] block of shape [85, 32*4] -> out [128, 84] in PSUM)
and DMA'd PSUM->DRAM. Audio enters via one DMA per 32-step block straight
into partition 84 of the stored block, so no per-step engine copy is needed.

Host-side numpy does all layout shuffles (transposes are free there).
"""
import os
from contextlib import ExitStack

import numpy as np

import concourse.bass as bass
import concourse.tile as tile
from concourse import mybir
from concourse.bass import ds
from concourse.bass_utils import run_bass_kernel_spmd

F32 = mybir.dt.float32
ALU = mybir.AluOpType
AF = mybir.ActivationFunctionType

N_CORES = 8
B_TOT, T, C = 32, 8192, 84
BL = B_TOT // N_CORES          # 4 batch rows per core
SB = 32                        # steps per y-matmul block
NJ = 4                         # y-blocks per loop body
BS = SB * NJ                   # 128 steps per loop body
NB = T // BS                   # 64 loop iterations
SAMPLE_RATE_HZ = 48000.0


# ----------------------------------------------------------------- host math
def _coeffs(hfdc, zero_ratio, min_zeta, max_zeta, erb_break, erb_q):
    # pole_freqs with float32 stepping to mirror the jax f32 scan exactly
    pf = np.empty(C, np.float64)
    f = np.float32(20400.0)
    eb = np.float32(erb_break)
    eq = np.float32(erb_q)
    half = np.float32(0.5)
    for i in range(C):
        pf[i] = f
        f = np.float32(f - half * (eb + f) / eq)
    hfdc, zero_ratio, min_zeta, max_zeta, erb_break, erb_q = (
        float(hfdc), float(zero_ratio), float(min_zeta), float(max_zeta),
        float(erb_break), float(erb_q))
    f_ratio = zero_ratio ** 2 - 1.0
    x = pf * 2.0 / SAMPLE_RATE_HZ
    theta = x * np.pi
    a0 = np.cos(theta)
    c0 = np.sin(theta)
    zr1 = np.pi * (x - hfdc * x ** 3)
    r1 = 1.0 - zr1 * max_zeta
    erb_pf = (erb_break + pf) / erb_q
    min_zetas = min_zeta + 0.25 * (erb_pf / pf - min_zeta)
    zr = zr1 * (max_zeta - min_zetas)
    rpz = r1 + zr
    h = c0 * f_ratio
    g0 = (1.0 - 2.0 * rpz * a0 + rpz ** 2) / (
        1.0 - 2.0 * rpz * a0 + h * rpz * c0 + rpz ** 2)
    return a0, c0, r1, zr, h, g0


def _weights(a0, c0, h, g0):
    L = np.zeros((C, C))
    for i in range(C):
        L[i, i] = g0[i]
        if i:
            L[i, :i] = g0[i] * L[i - 1, :i]
    G = L[:, 0].copy()
    hc0 = h * c0
    ha0 = h * a0
    Wy_ru = np.tril(L) * hc0[None, :]
    Wy_rv = np.tril(L) * ha0[None, :]
    SL = np.zeros((C, C))
    SL[1:, :] = np.tril(L)[:-1, :]
    Wu_ru = np.diag(a0) + SL * hc0[None, :]
    Wu_rv = np.diag(-c0) + SL * ha0[None, :]
    wu_inp = np.empty(C)
    wu_inp[0] = 1.0
    wu_inp[1:] = G[:-1]
    # lhsT layouts [K, M]: K = [ru(0:84); inp(84)], M = out channel
    wu1t = np.zeros((C + 1, C), np.float32)
    wu1t[:C, :] = Wu_ru.T
    wu1t[C, :] = wu_inp
    wu2t = np.zeros((C + 1, C), np.float32)
    wu2t[:C, :] = Wu_rv.T
    # y-matmul rhs layouts [K, N=C]
    wy1t = np.zeros((C + 1, C), np.float32)
    wy1t[:C, :] = Wy_ru.T
    wy1t[C, :] = G
    wy2t = np.zeros((C + 1, C), np.float32)
    wy2t[:C, :] = Wy_rv.T
    return wu1t, wu2t, wy1t, wy2t


# ------------------------------------------------------------- device program
_CACHE = {}


def _build_program(legalize=True):
    if "nc" in _CACHE:
        return _CACHE["nc"]
    nc = bass.Bass("TRN2", target_bir_lowering=False, debug=False,
                   num_devices=N_CORES)
    aud = nc.dram_tensor("aud", [NB, NJ, SB * BL], F32,
                         kind="ExternalInput").ap()
    # packed constants: [85, 4*84 + 6 + 12 + 8] = weights | vecs | state0 | bcast
    NCON = 4 * C + 6 + 12 + 2 * BL
    con = nc.dram_tensor("con", [C + 1, NCON], F32, kind="ExternalInput").ap()
    yout = nc.dram_tensor("yout", [NB, NJ, SB * BL, C], F32,
                          kind="ExternalOutput").ap()

    with tile.TileContext(nc) as tc, ExitStack() as ctx:
        const = ctx.enter_context(tc.tile_pool(name="const", bufs=1))
        stp = ctx.enter_context(tc.tile_pool(name="stp", bufs=1))
        temps = ctx.enter_context(tc.tile_pool(name="temps", bufs=2))
        blkp = ctx.enter_context(tc.tile_pool(name="blkp", bufs=2))
        rvp = ctx.enter_context(tc.tile_pool(name="rvp", bufs=2))
        ups = ctx.enter_context(tc.tile_pool(name="ups", bufs=1, space="PSUM"))
        yps = ctx.enter_context(tc.tile_pool(name="yps", bufs=2, space="PSUM"))
        ysbp = ctx.enter_context(tc.tile_pool(name="ysbp", bufs=2))

        c_sb = const.tile([C + 1, NCON], F32)
        nc.sync.dma_start(c_sb[:, :], con)

        wu1t = c_sb[:, 0 * C:1 * C]
        wu2t = c_sb[0:C, 1 * C:2 * C]
        wy1t = c_sb[:, 2 * C:3 * C]
        wy2t = c_sb[0:C, 3 * C:4 * C]
        vbase = 4 * C
        vs_ap = c_sb[0:C, vbase + 0:vbase + 1]
        vo_ap = c_sb[0:C, vbase + 1:vbase + 2]
        zr_ap = c_sb[0:C, vbase + 2:vbase + 3]
        r1_ap = c_sb[0:C, vbase + 3:vbase + 4]
        c0_ap = c_sb[0:C, vbase + 4:vbase + 5]
        a0_ap = c_sb[0:C, vbase + 5:vbase + 6]
        sbase = vbase + 6
        s_sb = c_sb[0:C, sbase:sbase + 12]
        bbase = sbase + 12
        b_sb = c_sb[0:C, bbase:bbase + 2 * BL]

        u_ps = [ups.tile([C, BL], F32, name="u_ps0", tag="u_ps0"),
                ups.tile([C, BL], F32, name="u_ps1", tag="u_ps1")]
        v_sl = [stp.tile([C, BL], F32, name="v_sl0", tag="v_sl0"),
                stp.tile([C, BL], F32, name="v_sl1", tag="v_sl1")]
        # step s reads u(t-1) from u_ps[(s+1)%2], v(t-1) from v_sl[(s+1)%2],
        # v(t-2) from v_sl[s%2]; writes u(t)->u_ps[s%2], v(t)->v_sl[s%2].
        nc.vector.tensor_copy(u_ps[1][:, :], s_sb[:, 0:4])
        nc.vector.tensor_copy(v_sl[1][:, :], s_sb[:, 4:8])
        nc.vector.tensor_copy(v_sl[0][:, :], s_sb[:, 8:12])

        for i in range(NB):
            for j in range(NJ):
                ruext = blkp.tile([C + 1, SB, BL], F32, name="ruext", tag="ruext")
                rvst = rvp.tile([C, SB, BL], F32, name="rvst", tag="rvst")
                # audio for this 32-step block -> partition 84 of ruext
                nc.sync.dma_start(ruext[C:C + 1, :, :], aud[i, j, :])
                for ss in range(SB):
                    s = j * SB + ss
                    # state w = v / a0 (per-channel rescale, folded into consts)
                    wp = v_sl[(s + 1) % 2]   # w(t-1)
                    wpp = v_sl[s % 2]        # w(t-2)
                    up = u_ps[(s + 1) % 2]   # u(t-1)
                    uw = u_ps[s % 2]
                    wd = temps.tile([C, BL], F32, name="wd", tag="wd")
                    nc.vector.tensor_sub(wd[:, :], wp[:, :], wpp[:, :])
                    d = temps.tile([C, BL], F32, name="d", tag="d")
                    nc.vector.scalar_tensor_tensor(
                        d[:, :], wd[:, :], vs_ap, b_sb[:, 0:BL],
                        ALU.mult, ALU.add)
                    q = temps.tile([C, BL], F32, name="q", tag="q")
                    nc.vector.tensor_mul(q[:, :], d[:, :], d[:, :])
                    den = temps.tile([C, BL], F32, name="den", tag="den")
                    nc.vector.tensor_scalar(den[:, :], q[:, :], 1.0, None,
                                            ALU.add)
                    rec = temps.tile([C, BL], F32, name="rec", tag="rec")
                    nc.vector.reciprocal(rec[:, :], den[:, :])
                    r = temps.tile([C, BL], F32, name="r", tag="r")
                    nc.vector.scalar_tensor_tensor(
                        r[:, :], rec[:, :], zr_ap, b_sb[:, BL:2 * BL],
                        ALU.mult, ALU.add)
                    ru = ruext[0:C, ss, :]
                    nc.vector.tensor_mul(ru, r[:, :], up[:, :])
                    rv = rvst[:, ss, :]
                    nc.vector.tensor_mul(rv, r[:, :], wp[:, :])
                    t9 = temps.tile([C, BL], F32, name="t9", tag="t9")
                    nc.vector.tensor_scalar(t9[:, :], ru, c0_ap, None,
                                            ALU.mult)
                    # v(t) = a0*rv + c0*ru, overwrites v(t-2) slot
                    nc.vector.scalar_tensor_tensor(wpp[:, :], rv, a0_ap,
                                                   t9[:, :], ALU.mult,
                                                   ALU.add)
                    nc.tensor.matmul(uw[:, :], wu1t, ruext[:, ss, :],
                                     start=True, stop=False)
                    nc.tensor.matmul(uw[:, :], wu2t, rv,
                                     start=False, stop=True)
                yb = yps.tile([SB * BL, C], F32, name="yb", tag="yb")
                nc.tensor.matmul(yb[:, :], ruext[:, :, :], wy1t,
                                 start=True, stop=False)
                nc.tensor.matmul(yb[:, :], rvst[:, :, :], wy2t,
                                 start=False, stop=True)
                ysb = ysbp.tile([SB * BL, C], F32, name="ysb", tag="ysb")
                nc.scalar.copy(ysb[:, :], yb[:, :])
                nc.sync.dma_start(yout[i, j, :, :], ysb[:, :])

    if legalize:
        _legalize_waits(nc)
    _CACHE["nc"] = nc
    return nc


def _legalize_waits(nc, max_waits=2):
    """Walrus encodes at most 2 sync-wait commands per instruction; offload
    excess waits onto same-engine NoOps inserted right before (engine queues
    are in-order, so the NoOp's wait gates everything after it)."""
    import bass_rust
    nid = [0]
    for f in nc.m.functions:
        for blk in f.blocks:
            out = []
            changed = False
            for ins in blk.instructions:
                si = ins.sync_info
                waits = list(si.on_wait) if si is not None else []
                max_waits = 1
                if len(waits) > max_waits:
                    # keep the last max_waits (self-sem + closest producer);
                    # offload older ones, one per NoOp to stay conservative.
                    excess, keep = waits[:-max_waits], waits[-max_waits:]
                    for w in excess:
                        nop = mybir.InstNoOp(name=f"waitnop_{nid[0]}")
                        nid[0] += 1
                        nop.engine = ins.engine
                        nop.sync_info = bass_rust.SyncInfo(
                            on_wait=[w], on_update=[])
                        out.append(nop)
                    ins.sync_info = bass_rust.SyncInfo(
                        on_wait=keep, on_update=list(si.on_update))
                    changed = True
                out.append(ins)
            if changed:
                blk.instructions = out


# ------------------------------------------------------------------ interface
def _pack_inputs(audio, u0, v0, pv0, hfdc, zero_ratio, min_zeta, max_zeta,
                 erb_break, erb_q, v_offset, velocity_scale):
    a0, c0, r1, zr, h, g0 = _coeffs(hfdc, zero_ratio, min_zeta, max_zeta,
                                    erb_break, erb_q)
    wu1t, wu2t, wy1t, wy2t = _weights(a0, c0, h, g0)
    wts = np.concatenate([wu1t, wu2t, wy1t, wy2t], axis=1)  # [C+1, 4C]
    vecs = np.zeros((C + 1, 6), np.float32)
    vecs[:C] = np.stack([
        np.full(C, float(velocity_scale)),
        np.full(C, float(v_offset)),
        zr, r1, c0, a0,
    ], axis=1).astype(np.float32)
    bcast = np.zeros((C + 1, 2 * BL), np.float32)
    bcast[:C] = np.concatenate([
        np.full((C, BL), float(v_offset)),
        np.broadcast_to(r1[:, None], (C, BL)),
    ], axis=1).astype(np.float32)

    in_maps = []
    for k in range(N_CORES):
        bsl = slice(k * BL, (k + 1) * BL)
        a = np.ascontiguousarray(audio[bsl], np.float32)   # [BL, T]
        aud = np.ascontiguousarray(
            a.T.reshape(NB, NJ, SB, BL).reshape(NB, NJ, SB * BL))
        st = np.zeros((C + 1, 12), np.float32)
        st[:C] = np.concatenate([u0[bsl].T, v0[bsl].T, pv0[bsl].T],
                                axis=1).astype(np.float32)
        con = np.concatenate([wts.astype(np.float32), vecs, st, bcast],
                             axis=1)                       # [C+1, NCON]
        in_maps.append({
            "aud": aud,
            "con": np.ascontiguousarray(con),
        })
    return in_maps


def _unpack_output(results):
    out = np.empty((B_TOT, T, C), np.float32)
    for k in range(N_CORES):
        y = results[k]["yout"]                       # [NB, NJ, SB*BL, C]
        y = y.reshape(NB, NJ, SB, BL, C).transpose(3, 0, 1, 2, 4)
        out[k * BL:(k + 1) * BL] = y.reshape(BL, T, C)
    return out


def run(inputs_kw, trace=False):
    nc = _build_program()
    in_maps = _pack_inputs(
        np.asarray(inputs_kw["audio"], np.float32),
        np.asarray(inputs_kw["u0"], np.float32),
        np.asarray(inputs_kw["v0"], np.float32),
        np.asarray(inputs_kw["pv0"], np.float32),
        inputs_kw["high_f_damping_compression"], inputs_kw["zero_ratio"],
        inputs_kw["min_zeta"], inputs_kw["max_zeta"],
        inputs_kw["erb_break_freq"], inputs_kw["erb_q"],
        inputs_kw["v_offset"], inputs_kw["velocity_scale"])
    res = run_bass_kernel_spmd(nc, in_maps, list(range(N_CORES)),
                               trace=trace)
    return _unpack_output(res.results), res


def kernel(**inputs):
    out, _ = run(inputs, trace=False)
    return out


# revision 25
# speedup vs baseline: 1.8329x; 1.3771x over previous
"""CARCell (CARFAC cascade) Trainium2 Bass kernel.

Strategy
--------
Data-parallel over batch: 32 rows -> 8 NeuronCores x 4 rows. Per core the
T=8192 time recurrence is sequential; per step everything lives in a
[C=84 partitions, B=4 free] layout so the nonlinear-damping chain runs as
~10 tiny VectorE ops with per-partition-scalar channel constants, and the
sequential 84-channel cascade is reformulated as constant-weight matmuls:

    y(t)  = Wy_ru @ ru + Wy_rv @ rv + G * inp          (output cascade)
    u(t)  = Wu_ru @ ru + Wu_rv @ rv + wu_inp * inp     (cascade + rotation)
    v(t)  = c0 * ru + a0 * rv                          (elementwise)

with L[i,j] = prod_{k=j..i} g0[k] the cascade propagation matrix,
precomputed on the host in float64. The u-update matmul runs per step
(state carried in PSUM); the y matmul is batched over 32 steps
(lhsT = stored [ru;inp] block of shape [85, 32*4] -> out [128, 84] in PSUM)
and DMA'd PSUM->DRAM. Audio enters via one DMA per 32-step block straight
into partition 84 of the stored block, so no per-step engine copy is needed.

Host-side numpy does all layout shuffles (transposes are free there).
"""
import os
from contextlib import ExitStack

import numpy as np

import concourse.bass as bass
import concourse.tile as tile
from concourse import mybir
from concourse.bass import ds
from concourse.bass_utils import run_bass_kernel_spmd

F32 = mybir.dt.float32
ALU = mybir.AluOpType
AF = mybir.ActivationFunctionType

N_CORES = 8
B_TOT, T, C = 32, 8192, 84
BL = B_TOT // N_CORES          # 4 batch rows per core
SB = 32                        # steps per y-matmul block
NJ = 4                         # y-blocks per loop body
BS = SB * NJ                   # 128 steps per loop body
NB = T // BS                   # 64 loop iterations
SAMPLE_RATE_HZ = 48000.0


# ----------------------------------------------------------------- host math
def _coeffs(hfdc, zero_ratio, min_zeta, max_zeta, erb_break, erb_q):
    # pole_freqs with float32 stepping to mirror the jax f32 scan exactly
    pf = np.empty(C, np.float64)
    f = np.float32(20400.0)
    eb = np.float32(erb_break)
    eq = np.float32(erb_q)
    half = np.float32(0.5)
    for i in range(C):
        pf[i] = f
        f = np.float32(f - half * (eb + f) / eq)
    hfdc, zero_ratio, min_zeta, max_zeta, erb_break, erb_q = (
        float(hfdc), float(zero_ratio), float(min_zeta), float(max_zeta),
        float(erb_break), float(erb_q))
    f_ratio = zero_ratio ** 2 - 1.0
    x = pf * 2.0 / SAMPLE_RATE_HZ
    theta = x * np.pi
    a0 = np.cos(theta)
    c0 = np.sin(theta)
    zr1 = np.pi * (x - hfdc * x ** 3)
    r1 = 1.0 - zr1 * max_zeta
    erb_pf = (erb_break + pf) / erb_q
    min_zetas = min_zeta + 0.25 * (erb_pf / pf - min_zeta)
    zr = zr1 * (max_zeta - min_zetas)
    rpz = r1 + zr
    h = c0 * f_ratio
    g0 = (1.0 - 2.0 * rpz * a0 + rpz ** 2) / (
        1.0 - 2.0 * rpz * a0 + h * rpz * c0 + rpz ** 2)
    return a0, c0, r1, zr, h, g0


def _weights(a0, c0, h, g0, vs):
    L = np.zeros((C, C))
    for i in range(C):
        L[i, i] = g0[i]
        if i:
            L[i, :i] = g0[i] * L[i - 1, :i]
    G = L[:, 0].copy()
    hc0 = h * c0
    ha0 = h * a0
    Wy_ru = np.tril(L) * hc0[None, :]
    Wy_rv = np.tril(L) * ha0[None, :]
    SL = np.zeros((C, C))
    SL[1:, :] = np.tril(L)[:-1, :]
    Wu_ru = np.diag(a0) + SL * hc0[None, :]
    Wu_rv = np.diag(-c0) + SL * ha0[None, :]
    wu_inp = np.empty(C)
    wu_inp[0] = 1.0
    wu_inp[1:] = G[:-1]
    # lhsT layouts [K, M]: K = [ru(0:84); inp(84)], M = out channel
    wu1t = np.zeros((C + 1, C), np.float32)
    wu1t[:C, :] = Wu_ru.T
    wu1t[C, :] = wu_inp
    wu2t = np.zeros((C + 1, C), np.float32)
    wu2t[:C, :] = Wu_rv.T
    # y-matmul rhs layouts [K, N=C]
    wy1t = np.zeros((C + 1, C), np.float32)
    wy1t[:C, :] = Wy_ru.T
    wy1t[C, :] = G
    wy2t = np.zeros((C + 1, C), np.float32)
    wy2t[:C, :] = Wy_rv.T
    # z-state (z = vs*v): the stored rv-tensor is rz = vs*rv, so rz-side
    # weights absorb 1/vs; m-matmul computes m~ = vs*(c0*u + a0*v).
    wu2t[:C, :] /= vs
    wy2t[:C, :] /= vs
    Wm_ru = vs * (c0[:, None] * Wu_ru + np.diag(a0 * c0))
    Wm_rz = c0[:, None] * Wu_rv + np.diag(a0 * a0)
    wm1t = np.zeros((C + 1, C), np.float32)
    wm1t[:C, :] = Wm_ru.T
    wm1t[C, :] = vs * c0 * wu_inp
    wm2t = np.zeros((C + 1, C), np.float32)
    wm2t[:C, :] = Wm_rz.T
    return wu1t, wu2t, wy1t, wy2t, wm1t, wm2t


# ------------------------------------------------------------- device program
_CACHE = {}


def _build_program(legalize=True):
    if "nc" in _CACHE:
        return _CACHE["nc"]
    nc = bass.Bass("TRN2", target_bir_lowering=False, debug=False,
                   num_devices=N_CORES)
    aud = nc.dram_tensor("aud", [NB, NJ, SB * BL], F32,
                         kind="ExternalInput").ap()
    # packed constants: [85, 6*84 + 6 + 16 + 8] = weights | vecs | state0 | bcast
    NCON = 6 * C + 6 + 16 + 2 * BL
    con = nc.dram_tensor("con", [C + 1, NCON], F32, kind="ExternalInput").ap()
    yout = nc.dram_tensor("yout", [NB, NJ, SB * BL, C], F32,
                          kind="ExternalOutput").ap()

    with tile.TileContext(nc) as tc, ExitStack() as ctx:
        const = ctx.enter_context(tc.tile_pool(name="const", bufs=1))
        stp = ctx.enter_context(tc.tile_pool(name="stp", bufs=1))
        temps = ctx.enter_context(tc.tile_pool(name="temps", bufs=2))
        blkp = ctx.enter_context(tc.tile_pool(name="blkp", bufs=2))
        rvp = ctx.enter_context(tc.tile_pool(name="rvp", bufs=2))
        ups = ctx.enter_context(tc.tile_pool(name="ups", bufs=1, space="PSUM"))
        yps = ctx.enter_context(tc.tile_pool(name="yps", bufs=2, space="PSUM"))
        ysbp = ctx.enter_context(tc.tile_pool(name="ysbp", bufs=2))

        c_sb = const.tile([C + 1, NCON], F32)
        nc.sync.dma_start(c_sb[:, :], con)

        wu1t = c_sb[:, 0 * C:1 * C]
        wu2t = c_sb[0:C, 1 * C:2 * C]
        wy1t = c_sb[:, 2 * C:3 * C]
        wy2t = c_sb[0:C, 3 * C:4 * C]
        wm1t = c_sb[:, 4 * C:5 * C]
        wm2t = c_sb[0:C, 5 * C:6 * C]
        vbase = 6 * C
        vs_ap = c_sb[0:C, vbase + 0:vbase + 1]
        vo_ap = c_sb[0:C, vbase + 1:vbase + 2]
        zr_ap = c_sb[0:C, vbase + 2:vbase + 3]
        r1_ap = c_sb[0:C, vbase + 3:vbase + 4]
        c0_ap = c_sb[0:C, vbase + 4:vbase + 5]
        a0_ap = c_sb[0:C, vbase + 5:vbase + 6]
        sbase = vbase + 6
        s_sb = c_sb[0:C, sbase:sbase + 16]
        bbase = sbase + 16
        b_sb = c_sb[0:C, bbase:bbase + 2 * BL]

        u_ps = [ups.tile([C, BL], F32, name="u_ps0", tag="u_ps0"),
                ups.tile([C, BL], F32, name="u_ps1", tag="u_ps1")]
        m_ps = [ups.tile([C, BL], F32, name="m_ps0", tag="m_ps0"),
                ups.tile([C, BL], F32, name="m_ps1", tag="m_ps1")]
        v_sl = [stp.tile([C, BL], F32, name="v_sl0", tag="v_sl0"),
                stp.tile([C, BL], F32, name="v_sl1", tag="v_sl1")]
        # step s reads u(t-1) from u_ps[(s+1)%2], v(t-1) from v_sl[(s+1)%2],
        # v(t-2) from v_sl[s%2]; writes u(t)->u_ps[s%2], v(t)->v_sl[s%2].
        nc.vector.tensor_copy(u_ps[1][:, :], s_sb[:, 0:4])
        nc.vector.tensor_copy(v_sl[1][:, :], s_sb[:, 4:8])
        nc.vector.tensor_copy(v_sl[0][:, :], s_sb[:, 8:12])
        nc.vector.tensor_copy(m_ps[1][:, :], s_sb[:, 12:16])

        for i in range(NB):
            for j in range(NJ):
                ruext = blkp.tile([C + 1, SB, BL], F32, name="ruext", tag="ruext")
                rvst = rvp.tile([C, SB, BL], F32, name="rvst", tag="rvst")
                # audio for this 32-step block -> partition 84 of ruext
                nc.sync.dma_start(ruext[C:C + 1, :, :], aud[i, j, :])
                for ss in range(SB):
                    s = j * SB + ss
                    # state z = vs*v; m~ = vs*(c0*u + a0*v) from TensorE
                    zp = v_sl[(s + 1) % 2]   # z(t-1)
                    zpp = v_sl[s % 2]        # z(t-2)
                    up = u_ps[(s + 1) % 2]   # u(t-1)
                    uw = u_ps[s % 2]
                    mp = m_ps[(s + 1) % 2]   # m~(t-1)
                    mw = m_ps[s % 2]
                    d = temps.tile([C, BL], F32, name="d", tag="d")
                    nc.vector.scalar_tensor_tensor(
                        d[:, :], zp[:, :], vo_ap, zpp[:, :],
                        ALU.add, ALU.subtract)
                    q = temps.tile([C, BL], F32, name="q", tag="q")
                    nc.vector.tensor_mul(q[:, :], d[:, :], d[:, :])
                    den = temps.tile([C, BL], F32, name="den", tag="den")
                    nc.vector.tensor_scalar(den[:, :], q[:, :], 1.0, None,
                                            ALU.add)
                    rec = temps.tile([C, BL], F32, name="rec", tag="rec")
                    nc.vector.reciprocal(rec[:, :], den[:, :])
                    r = temps.tile([C, BL], F32, name="r", tag="r")
                    nc.vector.scalar_tensor_tensor(
                        r[:, :], rec[:, :], zr_ap, b_sb[:, BL:2 * BL],
                        ALU.mult, ALU.add)
                    ru = ruext[0:C, ss, :]
                    nc.vector.tensor_mul(ru, r[:, :], up[:, :])
                    rz = rvst[:, ss, :]
                    nc.vector.tensor_mul(rz, r[:, :], zp[:, :])
                    # z(t) = r * m~(t-1), overwrites z(t-2) slot
                    nc.vector.tensor_mul(zpp[:, :], r[:, :], mp[:, :])
                    nc.tensor.matmul(uw[:, :], wu1t, ruext[:, ss, :],
                                     start=True, stop=False)
                    nc.tensor.matmul(uw[:, :], wu2t, rz,
                                     start=False, stop=True)
                    nc.tensor.matmul(mw[:, :], wm1t, ruext[:, ss, :],
                                     start=True, stop=False)
                    nc.tensor.matmul(mw[:, :], wm2t, rz,
                                     start=False, stop=True)
                yb = yps.tile([SB * BL, C], F32, name="yb", tag="yb")
                nc.tensor.matmul(yb[:, :], ruext[:, :, :], wy1t,
                                 start=True, stop=False)
                nc.tensor.matmul(yb[:, :], rvst[:, :, :], wy2t,
                                 start=False, stop=True)
                ysb = ysbp.tile([SB * BL, C], F32, name="ysb", tag="ysb")
                nc.scalar.copy(ysb[:, :], yb[:, :])
                nc.sync.dma_start(yout[i, j, :, :], ysb[:, :])

    if legalize:
        _legalize_waits(nc)
    _CACHE["nc"] = nc
    return nc


def _legalize_waits(nc, max_waits=2):
    """Walrus encodes at most 2 sync-wait commands per instruction; offload
    excess waits onto same-engine NoOps inserted right before (engine queues
    are in-order, so the NoOp's wait gates everything after it)."""
    import bass_rust
    nid = [0]
    for f in nc.m.functions:
        for blk in f.blocks:
            out = []
            changed = False
            for ins in blk.instructions:
                si = ins.sync_info
                waits = list(si.on_wait) if si is not None else []
                max_waits = 1
                if len(waits) > max_waits:
                    # keep the last max_waits (self-sem + closest producer);
                    # offload older ones, one per NoOp to stay conservative.
                    excess, keep = waits[:-max_waits], waits[-max_waits:]
                    for w in excess:
                        nop = mybir.InstNoOp(name=f"waitnop_{nid[0]}")
                        nid[0] += 1
                        nop.engine = ins.engine
                        nop.sync_info = bass_rust.SyncInfo(
                            on_wait=[w], on_update=[])
                        out.append(nop)
                    ins.sync_info = bass_rust.SyncInfo(
                        on_wait=keep, on_update=list(si.on_update))
                    changed = True
                out.append(ins)
            if changed:
                blk.instructions = out


# ------------------------------------------------------------------ interface
def _pack_inputs(audio, u0, v0, pv0, hfdc, zero_ratio, min_zeta, max_zeta,
                 erb_break, erb_q, v_offset, velocity_scale):
    a0, c0, r1, zr, h, g0 = _coeffs(hfdc, zero_ratio, min_zeta, max_zeta,
                                    erb_break, erb_q)
    vs = float(velocity_scale)
    wu1t, wu2t, wy1t, wy2t, wm1t, wm2t = _weights(a0, c0, h, g0, vs)
    wts = np.concatenate([wu1t, wu2t, wy1t, wy2t, wm1t, wm2t],
                         axis=1)                           # [C+1, 6C]
    vecs = np.zeros((C + 1, 6), np.float32)
    vecs[:C] = np.stack([
        np.full(C, float(velocity_scale)),
        np.full(C, float(v_offset)),
        zr, r1, c0, a0,
    ], axis=1).astype(np.float32)
    bcast = np.zeros((C + 1, 2 * BL), np.float32)
    bcast[:C] = np.concatenate([
        np.full((C, BL), float(v_offset)),
        np.broadcast_to(r1[:, None], (C, BL)),
    ], axis=1).astype(np.float32)

    in_maps = []
    for k in range(N_CORES):
        bsl = slice(k * BL, (k + 1) * BL)
        a = np.ascontiguousarray(audio[bsl], np.float32)   # [BL, T]
        aud = np.ascontiguousarray(
            a.T.reshape(NB, NJ, SB, BL).reshape(NB, NJ, SB * BL))
        st = np.zeros((C + 1, 16), np.float32)
        m0 = vs * (c0[:, None] * u0[bsl].T + a0[:, None] * v0[bsl].T)
        st[:C] = np.concatenate([u0[bsl].T, vs * v0[bsl].T,
                                 vs * pv0[bsl].T, m0],
                                axis=1).astype(np.float32)
        con = np.concatenate([wts.astype(np.float32), vecs, st, bcast],
                             axis=1)                       # [C+1, NCON]
        in_maps.append({
            "aud": aud,
            "con": np.ascontiguousarray(con),
        })
    return in_maps


def _unpack_output(results):
    out = np.empty((B_TOT, T, C), np.float32)
    for k in range(N_CORES):
        y = results[k]["yout"]                       # [NB, NJ, SB*BL, C]
        y = y.reshape(NB, NJ, SB, BL, C).transpose(3, 0, 1, 2, 4)
        out[k * BL:(k + 1) * BL] = y.reshape(BL, T, C)
    return out


def run(inputs_kw, trace=False):
    nc = _build_program()
    in_maps = _pack_inputs(
        np.asarray(inputs_kw["audio"], np.float32),
        np.asarray(inputs_kw["u0"], np.float32),
        np.asarray(inputs_kw["v0"], np.float32),
        np.asarray(inputs_kw["pv0"], np.float32),
        inputs_kw["high_f_damping_compression"], inputs_kw["zero_ratio"],
        inputs_kw["min_zeta"], inputs_kw["max_zeta"],
        inputs_kw["erb_break_freq"], inputs_kw["erb_q"],
        inputs_kw["v_offset"], inputs_kw["velocity_scale"])
    res = run_bass_kernel_spmd(nc, in_maps, list(range(N_CORES)),
                               trace=trace)
    return _unpack_output(res.results), res


def kernel(**inputs):
    out, _ = run(inputs, trace=False)
    return out


# revision 27
# speedup vs baseline: 1.8331x; 1.0001x over previous
"""CARCell (CARFAC cascade) Trainium2 Bass kernel.

Strategy
--------
Data-parallel over batch: 32 rows -> 8 NeuronCores x 4 rows. Per core the
T=8192 time recurrence is sequential; per step everything lives in a
[C=84 partitions, B=4 free] layout so the nonlinear-damping chain runs as
~10 tiny VectorE ops with per-partition-scalar channel constants, and the
sequential 84-channel cascade is reformulated as constant-weight matmuls:

    y(t)  = Wy_ru @ ru + Wy_rv @ rv + G * inp          (output cascade)
    u(t)  = Wu_ru @ ru + Wu_rv @ rv + wu_inp * inp     (cascade + rotation)
    v(t)  = c0 * ru + a0 * rv                          (elementwise)

with L[i,j] = prod_{k=j..i} g0[k] the cascade propagation matrix,
precomputed on the host in float64. The u-update matmul runs per step
(state carried in PSUM); the y matmul is batched over 32 steps
(lhsT = stored [ru;inp] block of shape [85, 32*4] -> out [128, 84] in PSUM)
and DMA'd PSUM->DRAM. Audio enters via one DMA per 32-step block straight
into partition 84 of the stored block, so no per-step engine copy is needed.

Host-side numpy does all layout shuffles (transposes are free there).
"""
import os
from contextlib import ExitStack

import numpy as np

import concourse.bass as bass
import concourse.tile as tile
from concourse import mybir
from concourse.bass import ds
from concourse.bass_utils import run_bass_kernel_spmd

F32 = mybir.dt.float32
ALU = mybir.AluOpType
AF = mybir.ActivationFunctionType

N_CORES = 8
B_TOT, T, C = 32, 8192, 84
BL = B_TOT // N_CORES          # 4 batch rows per core
SB = 32                        # steps per y-matmul block
NJ = 4                         # y-blocks per loop body
BS = SB * NJ                   # 128 steps per loop body
NB = T // BS                   # 64 loop iterations
SAMPLE_RATE_HZ = 48000.0


# ----------------------------------------------------------------- host math
def _coeffs(hfdc, zero_ratio, min_zeta, max_zeta, erb_break, erb_q):
    # pole_freqs with float32 stepping to mirror the jax f32 scan exactly
    pf = np.empty(C, np.float64)
    f = np.float32(20400.0)
    eb = np.float32(erb_break)
    eq = np.float32(erb_q)
    half = np.float32(0.5)
    for i in range(C):
        pf[i] = f
        f = np.float32(f - half * (eb + f) / eq)
    hfdc, zero_ratio, min_zeta, max_zeta, erb_break, erb_q = (
        float(hfdc), float(zero_ratio), float(min_zeta), float(max_zeta),
        float(erb_break), float(erb_q))
    f_ratio = zero_ratio ** 2 - 1.0
    x = pf * 2.0 / SAMPLE_RATE_HZ
    theta = x * np.pi
    a0 = np.cos(theta)
    c0 = np.sin(theta)
    zr1 = np.pi * (x - hfdc * x ** 3)
    r1 = 1.0 - zr1 * max_zeta
    erb_pf = (erb_break + pf) / erb_q
    min_zetas = min_zeta + 0.25 * (erb_pf / pf - min_zeta)
    zr = zr1 * (max_zeta - min_zetas)
    rpz = r1 + zr
    h = c0 * f_ratio
    g0 = (1.0 - 2.0 * rpz * a0 + rpz ** 2) / (
        1.0 - 2.0 * rpz * a0 + h * rpz * c0 + rpz ** 2)
    return a0, c0, r1, zr, h, g0


def _weights(a0, c0, h, g0, vs):
    L = np.zeros((C, C))
    for i in range(C):
        L[i, i] = g0[i]
        if i:
            L[i, :i] = g0[i] * L[i - 1, :i]
    G = L[:, 0].copy()
    hc0 = h * c0
    ha0 = h * a0
    Wy_ru = np.tril(L) * hc0[None, :]
    Wy_rv = np.tril(L) * ha0[None, :]
    SL = np.zeros((C, C))
    SL[1:, :] = np.tril(L)[:-1, :]
    Wu_ru = np.diag(a0) + SL * hc0[None, :]
    Wu_rv = np.diag(-c0) + SL * ha0[None, :]
    wu_inp = np.empty(C)
    wu_inp[0] = 1.0
    wu_inp[1:] = G[:-1]
    # lhsT layouts [K, M]: K = [ru(0:84); inp(84)], M = out channel
    wu1t = np.zeros((C + 1, C), np.float32)
    wu1t[:C, :] = Wu_ru.T
    wu1t[C, :] = wu_inp
    wu2t = np.zeros((C + 1, C), np.float32)
    wu2t[:C, :] = Wu_rv.T
    # y-matmul rhs layouts [K, N=C]
    wy1t = np.zeros((C + 1, C), np.float32)
    wy1t[:C, :] = Wy_ru.T
    wy1t[C, :] = G
    wy2t = np.zeros((C + 1, C), np.float32)
    wy2t[:C, :] = Wy_rv.T
    # z-state (z = vs*v): the stored rv-tensor is rz = vs*rv, so rz-side
    # weights absorb 1/vs; m-matmul computes m~ = vs*(c0*u + a0*v).
    wu2t[:C, :] /= vs
    wy2t[:C, :] /= vs
    Wm_ru = vs * (c0[:, None] * Wu_ru + np.diag(a0 * c0))
    Wm_rz = c0[:, None] * Wu_rv + np.diag(a0 * a0)
    wm1t = np.zeros((C + 1, C), np.float32)
    wm1t[:C, :] = Wm_ru.T
    wm1t[C, :] = vs * c0 * wu_inp
    wm2t = np.zeros((C + 1, C), np.float32)
    wm2t[:C, :] = Wm_rz.T
    return wu1t, wu2t, wy1t, wy2t, wm1t, wm2t


# ------------------------------------------------------------- device program
_CACHE = {}


def _build_program(legalize=True):
    if "nc" in _CACHE:
        return _CACHE["nc"]
    nc = bass.Bass("TRN2", target_bir_lowering=False, debug=False,
                   num_devices=N_CORES)
    aud = nc.dram_tensor("aud", [NB, NJ, SB * BL], F32,
                         kind="ExternalInput").ap()
    # packed constants: [85, 6*84 + 6 + 16 + 8] = weights | vecs | state0 | bcast
    NCON = 6 * C + 6 + 16 + 2 * BL
    con = nc.dram_tensor("con", [C + 1, NCON], F32, kind="ExternalInput").ap()
    yout = nc.dram_tensor("yout", [NB, NJ, SB * BL, C], F32,
                          kind="ExternalOutput").ap()

    with tile.TileContext(nc) as tc, ExitStack() as ctx:
        const = ctx.enter_context(tc.tile_pool(name="const", bufs=1))
        stp = ctx.enter_context(tc.tile_pool(name="stp", bufs=1))
        temps = ctx.enter_context(tc.tile_pool(name="temps", bufs=2))
        blkp = ctx.enter_context(tc.tile_pool(name="blkp", bufs=2))
        rvp = ctx.enter_context(tc.tile_pool(name="rvp", bufs=2))
        ups = ctx.enter_context(tc.tile_pool(name="ups", bufs=1, space="PSUM"))
        yps = ctx.enter_context(tc.tile_pool(name="yps", bufs=2, space="PSUM"))
        ysbp = ctx.enter_context(tc.tile_pool(name="ysbp", bufs=2))

        c_sb = const.tile([C + 1, NCON], F32)
        nc.sync.dma_start(c_sb[:, :], con)

        wu1t = c_sb[:, 0 * C:1 * C]
        wu2t = c_sb[0:C, 1 * C:2 * C]
        wy1t = c_sb[:, 2 * C:3 * C]
        wy2t = c_sb[0:C, 3 * C:4 * C]
        wm1t = c_sb[:, 4 * C:5 * C]
        wm2t = c_sb[0:C, 5 * C:6 * C]
        vbase = 6 * C
        vs_ap = c_sb[0:C, vbase + 0:vbase + 1]
        vo_ap = c_sb[0:C, vbase + 1:vbase + 2]
        zr_ap = c_sb[0:C, vbase + 2:vbase + 3]
        r1_ap = c_sb[0:C, vbase + 3:vbase + 4]
        c0_ap = c_sb[0:C, vbase + 4:vbase + 5]
        a0_ap = c_sb[0:C, vbase + 5:vbase + 6]
        sbase = vbase + 6
        s_sb = c_sb[0:C, sbase:sbase + 16]
        bbase = sbase + 16
        b_sb = c_sb[0:C, bbase:bbase + 2 * BL]

        u_ps = [ups.tile([C, BL], F32, name="u_ps0", tag="u_ps0"),
                ups.tile([C, BL], F32, name="u_ps1", tag="u_ps1")]
        m_ps = [ups.tile([C, BL], F32, name="m_ps0", tag="m_ps0"),
                ups.tile([C, BL], F32, name="m_ps1", tag="m_ps1")]
        v_sl = [stp.tile([C, BL], F32, name="v_sl0", tag="v_sl0"),
                stp.tile([C, BL], F32, name="v_sl1", tag="v_sl1")]
        # step s reads u(t-1) from u_ps[(s+1)%2], v(t-1) from v_sl[(s+1)%2],
        # v(t-2) from v_sl[s%2]; writes u(t)->u_ps[s%2], v(t)->v_sl[s%2].
        nc.vector.tensor_copy(u_ps[1][:, :], s_sb[:, 0:4])
        nc.vector.tensor_copy(v_sl[1][:, :], s_sb[:, 4:8])
        nc.vector.tensor_copy(v_sl[0][:, :], s_sb[:, 8:12])
        nc.vector.tensor_copy(m_ps[1][:, :], s_sb[:, 12:16])

        for i in range(NB):
            for j in range(NJ):
                ruext = blkp.tile([C + 1, SB, BL], F32, name="ruext", tag="ruext")
                rvst = rvp.tile([C, SB, BL], F32, name="rvst", tag="rvst")
                # audio for this 32-step block -> partition 84 of ruext
                nc.sync.dma_start(ruext[C:C + 1, :, :], aud[i, j, :])
                for ss in range(SB):
                    s = j * SB + ss
                    # state z = vs*v; m~ = vs*(c0*u + a0*v) from TensorE
                    zp = v_sl[(s + 1) % 2]   # z(t-1)
                    zpp = v_sl[s % 2]        # z(t-2)
                    up = u_ps[(s + 1) % 2]   # u(t-1)
                    uw = u_ps[s % 2]
                    mp = m_ps[(s + 1) % 2]   # m~(t-1)
                    mw = m_ps[s % 2]
                    d = temps.tile([C, BL], F32, name="d", tag="d")
                    nc.vector.scalar_tensor_tensor(
                        d[:, :], zp[:, :], vo_ap, zpp[:, :],
                        ALU.add, ALU.subtract)
                    q = temps.tile([C, BL], F32, name="q", tag="q")
                    nc.vector.tensor_mul(q[:, :], d[:, :], d[:, :])
                    den = temps.tile([C, BL], F32, name="den", tag="den")
                    nc.vector.tensor_scalar(den[:, :], q[:, :], 1.0, None,
                                            ALU.add)
                    rec = temps.tile([C, BL], F32, name="rec", tag="rec")
                    nc.vector.reciprocal(rec[:, :], den[:, :])
                    r = temps.tile([C, BL], F32, name="r", tag="r")
                    nc.vector.scalar_tensor_tensor(
                        r[:, :], rec[:, :], zr_ap, b_sb[:, BL:2 * BL],
                        ALU.mult, ALU.add)
                    ru = ruext[0:C, ss, :]
                    nc.vector.tensor_mul(ru, r[:, :], up[:, :])
                    rz = rvst[:, ss, :]
                    nc.vector.tensor_mul(rz, r[:, :], zp[:, :])
                    # z(t) = r * m~(t-1), overwrites z(t-2) slot
                    nc.vector.tensor_mul(zpp[:, :], r[:, :], mp[:, :])
                    nc.tensor.matmul(uw[:, :], wu1t, ruext[:, ss, :],
                                     start=True, stop=False)
                    nc.tensor.matmul(uw[:, :], wu2t, rz,
                                     start=False, stop=True)
                    nc.tensor.matmul(mw[:, :], wm1t, ruext[:, ss, :],
                                     start=True, stop=False)
                    nc.tensor.matmul(mw[:, :], wm2t, rz,
                                     start=False, stop=True)
                yb = yps.tile([SB * BL, C], F32, name="yb", tag="yb")
                nc.tensor.matmul(yb[:, :], ruext[:, :, :], wy1t,
                                 start=True, stop=False)
                nc.tensor.matmul(yb[:, :], rvst[:, :, :], wy2t,
                                 start=False, stop=True)
                ysb = ysbp.tile([SB * BL, C], F32, name="ysb", tag="ysb")
                nc.scalar.copy(ysb[:, :], yb[:, :])
                nc.sync.dma_start(yout[i, j, :, :], ysb[:, :])

    if legalize:
        _legalize_waits(nc)
    _CACHE["nc"] = nc
    return nc


def _legalize_waits(nc, max_waits=2):
    """Walrus encodes at most 2 sync-wait commands per instruction; offload
    excess waits onto same-engine NoOps inserted right before (engine queues
    are in-order, so the NoOp's wait gates everything after it)."""
    import bass_rust
    nid = [0]
    for f in nc.m.functions:
        for blk in f.blocks:
            out = []
            changed = False
            for ins in blk.instructions:
                si = ins.sync_info
                waits = list(si.on_wait) if si is not None else []
                max_waits = 1
                if len(waits) > max_waits:
                    # keep the last max_waits (self-sem + closest producer);
                    # offload older ones, one per NoOp to stay conservative.
                    excess, keep = waits[:-max_waits], waits[-max_waits:]
                    for w in excess:
                        nop = mybir.InstNoOp(name=f"waitnop_{nid[0]}")
                        nid[0] += 1
                        nop.engine = ins.engine
                        nop.sync_info = bass_rust.SyncInfo(
                            on_wait=[w], on_update=[])
                        out.append(nop)
                    ins.sync_info = bass_rust.SyncInfo(
                        on_wait=keep, on_update=list(si.on_update))
                    changed = True
                out.append(ins)
            if changed:
                blk.instructions = out


# ------------------------------------------------------------------ interface
def _pack_inputs(audio, u0, v0, pv0, hfdc, zero_ratio, min_zeta, max_zeta,
                 erb_break, erb_q, v_offset, velocity_scale):
    a0, c0, r1, zr, h, g0 = _coeffs(hfdc, zero_ratio, min_zeta, max_zeta,
                                    erb_break, erb_q)
    vs = float(velocity_scale)
    wu1t, wu2t, wy1t, wy2t, wm1t, wm2t = _weights(a0, c0, h, g0, vs)
    wts = np.concatenate([wu1t, wu2t, wy1t, wy2t, wm1t, wm2t],
                         axis=1)                           # [C+1, 6C]
    vecs = np.zeros((C + 1, 6), np.float32)
    vecs[:C] = np.stack([
        np.full(C, float(velocity_scale)),
        np.full(C, float(v_offset)),
        zr, r1, c0, a0,
    ], axis=1).astype(np.float32)
    bcast = np.zeros((C + 1, 2 * BL), np.float32)
    bcast[:C] = np.concatenate([
        np.full((C, BL), float(v_offset)),
        np.broadcast_to(r1[:, None], (C, BL)),
    ], axis=1).astype(np.float32)

    in_maps = []
    for k in range(N_CORES):
        bsl = slice(k * BL, (k + 1) * BL)
        a = np.ascontiguousarray(audio[bsl], np.float32)   # [BL, T]
        aud = np.ascontiguousarray(
            a.T.reshape(NB, NJ, SB, BL).reshape(NB, NJ, SB * BL))
        st = np.zeros((C + 1, 16), np.float32)
        m0 = vs * (c0[:, None] * u0[bsl].T + a0[:, None] * v0[bsl].T)
        st[:C] = np.concatenate([u0[bsl].T, vs * v0[bsl].T,
                                 vs * pv0[bsl].T, m0],
                                axis=1).astype(np.float32)
        con = np.concatenate([wts.astype(np.float32), vecs, st, bcast],
                             axis=1)                       # [C+1, NCON]
        in_maps.append({
            "aud": aud,
            "con": np.ascontiguousarray(con),
        })
    return in_maps


def _unpack_output(results):
    out = np.empty((B_TOT, T, C), np.float32)
    for k in range(N_CORES):
        y = results[k]["yout"]                       # [NB, NJ, SB*BL, C]
        y = y.reshape(NB, NJ, SB, BL, C).transpose(3, 0, 1, 2, 4)
        out[k * BL:(k + 1) * BL] = y.reshape(BL, T, C)
    return out


def run(inputs_kw, trace=False):
    nc = _build_program()
    in_maps = _pack_inputs(
        np.asarray(inputs_kw["audio"], np.float32),
        np.asarray(inputs_kw["u0"], np.float32),
        np.asarray(inputs_kw["v0"], np.float32),
        np.asarray(inputs_kw["pv0"], np.float32),
        inputs_kw["high_f_damping_compression"], inputs_kw["zero_ratio"],
        inputs_kw["min_zeta"], inputs_kw["max_zeta"],
        inputs_kw["erb_break_freq"], inputs_kw["erb_q"],
        inputs_kw["v_offset"], inputs_kw["velocity_scale"])
    res = run_bass_kernel_spmd(nc, in_maps, list(range(N_CORES)),
                               trace=trace)
    return _unpack_output(res.results), res


def kernel(**inputs):
    out, _ = run(inputs, trace=False)
    return out
